# revision 23
# baseline (speedup 1.0000x reference)
"""MiMo V2 MoE gate (sigmoid routing, grouped top-k) on 8 Trainium2 cores.

Contract: kernel(**inputs) takes the FULL unsharded inputs
(hidden_states [4,4096,4096] f32, weight [256,4096] f32,
e_score_correction_bias [256] f32) and returns (topk_idx int32 [16384,8],
topk_weight f32 [16384,8]) matching reference.py.

Strategy (data-parallel over tokens):
  - 16384 tokens are sharded 2048/core across 8 NeuronCores.
  - Gate GEMM runs as a 3-pass bf16 split (x = xh + xl, w = wh + wl;
    logits = xh.wh + xh.wl + xl.wh, the dropped xl.wl term is ~2^-18
    relative) with fp32 PSUM accumulation: 3/4 the PE cycles of the exact
    fp32 4-pass mode at ~fp32 precision. (fp32r was measured on HW to
    round operands to a 12-bit significand - too coarse for the routing
    ties; bf16x3's residual split keeps ~17 bits.)
  - Host pre-packs each operand per (token-tile, partition) so every DMA
    line is one long contiguous run (8KB/partition/tile per operand) -
    the HWDGE hits full HBM bandwidth instead of 512B scatter lines.
  - Per 128-token tile: sigmoid on ScalarE; the grouped top-k on
    VectorE (sort8 ops: max / max_index / match_replace); elementwise
    mask/select work offloaded to GpSimd to keep the DVE off the
    critical path.
"""

import sys

if "/opt/trn_rl_repo" not in sys.path:
    sys.path.insert(0, "/opt/trn_rl_repo")

import numpy as np

import concourse.bass as bass
import concourse.mybir as mybir
import concourse.tile as tile
from concourse.tile_rust import add_dep_helper, annotate_deps

P = 128
H = 4096
E = 256
N_CORES = 8
T_FULL = 16384
T_CORE = T_FULL // N_CORES  # 2048
KC = H // P                 # 32 contraction chunks
TOK_TILES = T_CORE // P     # 16 token tiles per core
N_GROUP = 8
EG = E // N_GROUP           # 32 experts per group
TOPK_GROUP = 4
TOP_K = 8
ROUTED_SCALING = 2.5
NEG = -1e30

F32 = mybir.dt.float32
BF16 = mybir.dt.bfloat16
U32 = mybir.dt.uint32
AF = mybir.ActivationFunctionType
OP = mybir.AluOpType


def _reserve(nc, eng, X, n, prev=None):
    """Emit n plain nops on X's engine, ordered after `prev` (a
    BassInstruction or None) and before X. They act as spare 1-wait
    carriers for _legalize_waits (every TPB instruction has exactly one
    HW wait slot; Tile can assign several waits to one instruction,
    which walrus then rejects)."""
    last = prev.ins if prev is not None else None
    for _ in range(n):
        nop = eng.nop(nofuse=True)
        if last is not None:
            add_dep_helper(nop.ins, last, sync=False,
                           reason="chain reserve nop after predecessor")
        add_dep_helper(X.ins, nop.ins, sync=False,
                       reason="reserve nop precedes its instruction")
        last = nop.ins


def _legalize_waits(nc, report=None):
    """Every TPB instruction has ONE hardware wait slot; Tile can assign
    several on_wait entries to an instruction, which walrus rejects
    ("Too many sync wait commands"). Fix in two ways, per engine stream
    (scheduled order):
      1. value-floor dedup: drop waits already implied by an earlier wait
         on the same semaphore in this stream (monotonic sems).
      2. excess-wait hoisting: move extra waits onto the nearest earlier
         wait-free instruction, scanning only across instructions with no
         on_update (pure nops) -- crossing an updater could reorder a
         producer chain and deadlock; this rule keeps placements provably
         safe. _reserve() plants such nops next to risky instructions.
    Drains are skipped (they encode multi-sem waits natively)."""
    stop_types = (
        mybir.InstDrain,
        mybir.InstEventSemaphore,
        mybir.InstCall,
    )
    leftover = []
    if True:
        # The kernel CFG is linear (main block -> end block), so per-engine
        # program order is the block-order concatenation. Crossing an
        # unconditional branch just means waiting before the jump.
        streams = {}
        nonmono = set()  # sems that ever decrease (barrier sems): no
                         # floor-dedup and no relocation for their waits
        for blk in nc.m.functions[0].blocks:
            for inst in blk.instructions:
                streams.setdefault(str(inst.engine), []).append(inst)
                si = inst.sync_info
                for u in (si.on_update if si and si.on_update else []):
                    if str(u.update_mode) not in ('sem-inc', 'sem-add-imm'):
                        nonmono.add(u.id)
        for stream in streams.values():
            floor = {}
            for i, X in enumerate(stream):
                si = X.sync_info
                if si is None or not si.on_wait:
                    continue
                mode_ok = lambda w: (str(w.wait_mode) == 'sem-ge-imm'
                                     and w.id not in nonmono)
                waits = []
                for w in si.on_wait:
                    if (mode_ok(w) and w.id in floor
                            and floor[w.id] >= w.wait_value):
                        continue  # already implied earlier in this stream
                    waits.append(w)
                moved = []
                if len(waits) > 1:
                    # only sem-ge waits are relocatable; sem-sub barrier
                    # ops must stay exactly where Tile put them
                    fixed = [w for w in waits if not mode_ok(w)]
                    movable = [w for w in waits if mode_ok(w)]
                    keep = fixed + movable[:max(0, 1 - len(fixed))]
                    maybe_move = movable[max(0, 1 - len(fixed)):]
                    for w in maybe_move:
                        placed = False
                        for k in range(i - 1, -1, -1):
                            C = stream[k]
                            if isinstance(C, stop_types):
                                break
                            csi = C.sync_info
                            if csi and csi.on_update:
                                break  # never cross a semaphore producer
                            cw = list(csi.on_wait) if csi and csi.on_wait else []
                            if cw or isinstance(
                                    C, mybir.InstUnconditionalBranch):
                                continue  # occupied/branch; keep scanning
                                          # (same-sequencer waits commute)
                            C.sync_info = mybir.SyncInfo(on_wait=[w],
                                                         on_update=[])
                            placed = True
                            break
                        if placed:
                            moved.append(w)
                        else:
                            keep.append(w)
                    waits = keep
                for w in list(waits) + moved:
                    if mode_ok(w):
                        floor[w.id] = max(floor.get(w.id, 0), w.wait_value)
                X.sync_info = mybir.SyncInfo(
                    on_wait=waits,
                    on_update=list(si.on_update) if si.on_update else [])
                if len(waits) > 1:
                    leftover.append((X.name, str(X.engine),
                                     type(X).__name__, len(waits)))
    # The PE gate ENGINE_NOPs carry AP operands purely for Tile dep
    # tracking; walrus's engine check rejects a nop with operands, so
    # strip them now (tile.py does the same for InstNoOp instructions).
    for blk in nc.m.functions[0].blocks:
        for inst in blk.instructions:
            if (isinstance(inst, mybir.InstISA) and (inst.ins or inst.outs)
                    and inst.op_name == 'ENGINE_NOP'):
                inst.ins = []
                inst.outs = []

    if report is not None:
        report.extend(leftover)
    elif leftover:
        raise RuntimeError(f"wait legalization failed for: {leftover}")


def build_nc():
    nc = bass.Bass()

    # Host-packed layouts (see make_in_maps):
    #   xh/xl: [TOK_TILES, P, KC*P]  bf16 - per (tile, partition) one
    #          contiguous 8KB run (32 chunks x 128 tokens x 2B).
    #   wh/wl: [P, KC*E] bf16 - per partition one contiguous 16KB run.
    xh_d = nc.dram_tensor("xh", [TOK_TILES, P, KC * P], BF16, kind="ExternalInput")
    xl_d = nc.dram_tensor("xl", [TOK_TILES, P, KC * P], BF16, kind="ExternalInput")
    wh_d = nc.dram_tensor("wh", [P, KC * E], BF16, kind="ExternalInput")
    wl_d = nc.dram_tensor("wl", [P, KC * E], BF16, kind="ExternalInput")
    biasb = nc.dram_tensor("biasb", [P, E], F32, kind="ExternalInput")
    idx_out = nc.dram_tensor("idx_out", [T_CORE, TOP_K], U32, kind="ExternalOutput")
    w_out = nc.dram_tensor("w_out", [T_CORE, TOP_K], F32, kind="ExternalOutput")

    with tile.TileContext(nc) as tc:
        with (
            tc.tile_pool(name="const", bufs=1) as cpool,
            tc.tile_pool(name="xin", bufs=6) as xpool,
            tc.tile_pool(name="psum", bufs=2, space="PSUM") as pspool,
            tc.tile_pool(name="work", bufs=2) as wpool,
        ):
            # Tile-0 x DMAs go FIRST so the lead-in isn't serialized behind
            # the 4MB weight download; the weights are split into 4 sub-DMAs
            # per operand so the first matmuls (which only read the first
            # chunks) start as soon as those land.
            # Each DMA queue/engine only sustains ~20-25 GB/s; aggregate
            # bandwidth comes from keeping many queues busy, so every
            # 1MB x-tile transfer is split into XSPLIT round-robin'd
            # sub-DMAs.
            XSPLIT = 2
            XC = KC // XSPLIT

            def one_x_dma(src, tag, prev, nsplit=XSPLIT):
                xt = xpool.tile([P, KC, P], BF16, tag=tag)
                src3 = src.rearrange("p (c m) -> p c m", c=KC)
                ns = KC // nsplit
                for s in range(nsplit):
                    cs = slice(s * ns, (s + 1) * ns)
                    dma = nc.sync.dma_start(xt[:, cs, :], src3[:, cs, :])
                    _reserve(nc, nc.sync, dma, 3, prev=prev)
                    prev = dma
                return xt, prev

            def x_dmas(j, prev):
                xht, xh_dma = one_x_dma(xh_d.ap()[j], "xh", prev)
                xlt, xl_dma = one_x_dma(xl_d.ap()[j], "xl", xh_dma)
                return xht, xlt, xl_dma

            # Startup issue order mirrors the pass-major consumption order
            # of tile 0: xh0, wh, xl0, wl - the PE starts after just
            # xh0 + the first wh chunks have landed.
            xht0, xh0_dma = one_x_dma(xh_d.ap()[0], "xh", None, nsplit=4)
            whs = cpool.tile([P, KC, E], BF16)
            wls = cpool.tile([P, KC, E], BF16)
            wh3 = wh_d.ap().rearrange("p (c e) -> p c e", c=KC)
            wl3 = wl_d.ap().rearrange("p (c e) -> p c e", c=KC)
            WSPLIT = 8
            CS = KC // WSPLIT
            prev_dma = xh0_dma
            for s in range(WSPLIT):
                cs = slice(s * CS, (s + 1) * CS)
                dh = nc.sync.dma_start(whs[:, cs, :], wh3[:, cs, :])
                _reserve(nc, nc.sync, dh, 2, prev=prev_dma)
                prev_dma = dh
            xlt0, xl0_dma = one_x_dma(xl_d.ap()[0], "xl", prev_dma)
            prev_dma = xl0_dma
            for s in range(WSPLIT):
                cs = slice(s * CS, (s + 1) * CS)
                dl = nc.sync.dma_start(wls[:, cs, :], wl3[:, cs, :])
                _reserve(nc, nc.sync, dl, 2, prev=prev_dma)
                prev_dma = dl
            x0 = (xht0, xlt0, prev_dma)
            bsb = cpool.tile([P, E], F32)
            bdma = nc.sync.dma_start(bsb[:], biasb.ap())
            _reserve(nc, nc.sync, bdma, 2, prev=prev_dma)
            prev_dma = bdma
            # Persistent per-core output accumulators: no slot reuse, so
            # the producers of idx/w never wait on output DMAs.
            idx_all = cpool.tile([P, TOK_TILES, TOP_K], U32)
            w_all = cpool.tile([P, TOK_TILES, TOP_K], F32)

            prev_sig = None
            prev_mm = None
            prev_dve = None
            prev_gdma = None
            for j in range(TOK_TILES):
                # ---- gate GEMM: logits[128 tok, 256 exp] = 3-pass bf16 ----
                if j == 0:
                    xht, xlt, _ = x0
                else:
                    xht, xlt, prev_dma = x_dmas(j, prev_dma)
                ps = pspool.tile([P, E], F32, tag="ps")
                # The tile-leading matmul needs the x-DMA sems plus the
                # psum-slot-release sem, but a matmul only has one HW wait
                # slot in walrus codegen. Emit a PE NoOp that declares those
                # data deps (1-elem APs, registered via annotate_deps) so
                # Tile's per-engine clock absorbs all waits there; the
                # matmuls then follow wait-free in PE program order. Tile
                # strips APs from InstNoOp at lowering, so walrus only
                # sees a plain NOP.
                gate = nc.tensor.nop(nofuse=True)
                gate.ins.ins = [
                    nc.tensor.lower_ap(xht[0:1, 0, 0:1]),
                    nc.tensor.lower_ap(xlt[0:1, 0, 0:1]),
                ]
                gate.ins.outs = [nc.tensor.lower_ap(ps[0:1, 0:1])]
                annotate_deps(tc.dep_state, gate.ins, tc.shadow_memory,
                              tc._rust_ctx, nc.inst_map)
                _reserve(nc, nc.tensor, gate, 4, prev=prev_mm)
                # Pass-major order: all xh.wh first (needs only xh + wh in
                # SBUF), then xl.wh, then xh.wl - so the first matmuls of
                # tile 0 can start before xl/wl even arrive.
                for c in range(KC):
                    nc.tensor.matmul(
                        ps[:], lhsT=xht[:, c, :], rhs=whs[:, c, :],
                        start=(c == 0), stop=False,
                    )
                for c in range(KC):
                    nc.tensor.matmul(
                        ps[:], lhsT=xlt[:, c, :], rhs=whs[:, c, :],
                        start=False, stop=False,
                    )
                for c in range(KC):
                    mm = nc.tensor.matmul(
                        ps[:], lhsT=xht[:, c, :], rhs=wls[:, c, :],
                        start=False, stop=(c == KC - 1),
                    )
                prev_mm = mm

                # ---- scores (ScalarE) / biased scores ----
                scores = wpool.tile([P, E], F32, tag="scores")
                sig = nc.scalar.activation(scores[:], ps[:], AF.Sigmoid)
                _reserve(nc, nc.scalar, sig, 3, prev=prev_sig)
                prev_sig = sig
                sfc = wpool.tile([P, E], F32, tag="sfc")
                badd = nc.vector.tensor_add(sfc[:], scores[:], bsb[:])
                _reserve(nc, nc.vector, badd, 3, prev=prev_dve)
                sfc3 = sfc[:].rearrange("p (g e) -> p g e", g=N_GROUP)

                # ---- group scores: sum of top-2 per group of 32 (DVE) ----
                g3 = wpool.tile([P, N_GROUP, 8], F32, tag="g3")
                for g in range(N_GROUP):
                    nc.vector.max(g3[:, g, :], sfc[:, g * EG:(g + 1) * EG])
                gsum = wpool.tile([P, N_GROUP], F32, tag="gsum")
                nc.vector.tensor_add(gsum[:], g3[:, :, 0], g3[:, :, 1])

                # ---- pick top-4 groups; additive mask 0 / -BIG ----
                g8 = wpool.tile([P, 8], F32, tag="g8")
                nc.vector.max(g8[:], gsum[:])
                gneg = wpool.tile([P, N_GROUP], F32, tag="gneg")
                # (gsum < 4th-largest) * NEG -> 0 for kept groups, NEG else
                nc.vector.tensor_scalar(
                    gneg[:], gsum[:], g8[:, TOPK_GROUP - 1:TOPK_GROUP], NEG,
                    op0=OP.is_lt, op1=OP.mult,
                )

                # ---- masked biased scores; top-8 experts ----
                tmp = wpool.tile([P, E], F32, tag="tmp")
                tmp3 = tmp[:].rearrange("p (g e) -> p g e", g=N_GROUP)
                nc.vector.tensor_tensor(
                    tmp3, sfc3, gneg[:, :, None].to_broadcast([P, N_GROUP, EG]),
                    op=OP.add,
                )
                max8 = wpool.tile([P, 8], F32, tag="max8")
                nc.vector.max(max8[:], tmp[:])
                idx8 = idx_all[:, j, :]
                nc.vector.max_index(idx8, max8[:], tmp[:])

                # ---- selected-set mask via match_replace diff ----
                zap = wpool.tile([P, E], F32, tag="zap")
                mrep = nc.vector.match_replace(
                    zap[:], in_to_replace=max8[:], in_values=tmp[:], imm_value=NEG
                )
                prev_dve = mrep

                # ---- unbiased scores of the selected 8 ----
                # eqz = (tmp == zap): 1 for NOT-selected positions.
                # sm = eqz * NEG + scores: scores at selected, -1e30 else.
                eqz = wpool.tile([P, E], F32, tag="eqz")
                nc.vector.tensor_tensor(eqz[:], tmp[:], zap[:], op=OP.is_equal)
                sm = wpool.tile([P, E], F32, tag="sm")
                nc.vector.scalar_tensor_tensor(
                    sm[:], eqz[:], NEG, scores[:], op0=OP.mult, op1=OP.add,
                )

                # ---- sorted selected scores + their positions ----
                smax8 = wpool.tile([P, 8], F32, tag="smax8")
                nc.vector.max(smax8[:], sm[:])
                sidx8 = wpool.tile([P, 8], U32, tag="sidx8")
                nc.vector.max_index(sidx8[:], smax8[:], sm[:])

                # ---- reorder scores to biased-rank order:
                #      w8[k] = sum_j smax8[j] * (sidx8[j] == idx8[k]) ----
                idxf = wpool.tile([P, 8], F32, tag="idxf")
                nc.vector.tensor_copy(idxf[:], idx8)
                sidxf = wpool.tile([P, 8], F32, tag="sidxf")
                nc.vector.tensor_copy(sidxf[:], sidx8[:])
                eq = wpool.tile([P, 8, 8], F32, tag="eq")
                nc.vector.tensor_tensor(
                    eq[:],
                    idxf[:, :, None].to_broadcast([P, 8, 8]),
                    sidxf[:, None, :].to_broadcast([P, 8, 8]),
                    op=OP.is_equal,
                )
                # wprod = eq * smax8 (broadcast); den = sum of all 64 = sum w8
                wprod = wpool.tile([P, 8, 8], F32, tag="wprod")
                den = wpool.tile([P, 1], F32, tag="den")
                nc.vector.scalar_tensor_tensor(
                    wprod[:], eq[:], 1.0,
                    smax8[:, None, :].to_broadcast([P, 8, 8]),
                    op0=OP.mult, op1=OP.mult, accum_out=den[:],
                )
                w8 = wpool.tile([P, 8], F32, tag="w8")
                nc.vector.reduce_sum(w8[:], wprod[:], axis=mybir.AxisListType.X)
                # w = 2.5 * w8 / (den + 1e-20); den = sum of 8 sigmoids
                # >= ~1e-3 on any real input, so the 1e-20 is sub-ulp and
                # folds away exactly as in the reference.
                rden = wpool.tile([P, 1], F32, tag="rden")
                nc.vector.reciprocal(rden[:], den[:])
                prev_dve = nc.vector.tensor_scalar(
                    w_all[:, j, :], w8[:], rden[:], ROUTED_SCALING,
                    op0=OP.mult, op1=OP.mult,
                )

                # ---- per-tile output DMAs (tile j's rows are one 4KB
                # contiguous block in DRAM) so the kernel-tail drain only
                # waits on the last tile's small writes. Issued from the
                # otherwise-idle GpSimd queue: putting them on the SP
                # stream would park the next tiles' x-prefetch triggers
                # behind a wait on this tile's DVE chain and collapse the
                # pipeline. ----
                od1 = nc.gpsimd.dma_start(
                    idx_out.ap()[j * P:(j + 1) * P, :], idx_all[:, j, :])
                _reserve(nc, nc.gpsimd, od1, 2, prev=prev_gdma)
                od2 = nc.gpsimd.dma_start(
                    w_out.ap()[j * P:(j + 1) * P, :], w_all[:, j, :])
                _reserve(nc, nc.gpsimd, od2, 2, prev=od1)
                prev_gdma = od2

            # Tail carriers: Tile's kernel-tail drain on SP waits on every
            # DMA queue sem (~20 waits with the gpsimd output queues);
            # give the legalizer enough nops.
            tail = prev_dma.ins
            for _ in range(24):
                nop = nc.sync.nop(nofuse=True)
                add_dep_helper(nop.ins, tail, sync=False,
                               reason="tail drain wait carriers")
                tail = nop.ins

    _legalize_waits(nc)
    return nc


class _Runner:
    """Compile-once SPMD runner (mirrors bass2jax.run_bass_via_pjrt's
    multi-core path, but holds the jitted fn so repeated calls don't
    re-trace/re-jit; inputs can stay resident on device for timing)."""

    def __init__(self, nc):
        import jax
        from jax.experimental.shard_map import shard_map
        from jax.sharding import Mesh, NamedSharding, PartitionSpec

        from concourse import bass2jax

        bass2jax.install_neuronx_cc_hook()
        self._jax = jax
        self.nc = nc

        partition_name = (
            nc.partition_id_tensor.name if nc.partition_id_tensor else None
        )
        in_names, out_names, out_avals, zero_outs = [], [], [], []
        for alloc in nc.m.functions[0].allocations:
            if not isinstance(alloc, mybir.MemoryLocationSet):
                continue
            name = alloc.memorylocations[0].name
            if alloc.kind == "ExternalInput":
                if name != partition_name:
                    in_names.append(name)
            elif alloc.kind == "ExternalOutput":
                shape = tuple(alloc.tensor_shape)
                dtype = mybir.dt.np(alloc.dtype)
                out_names.append(name)
                out_avals.append(jax.core.ShapedArray(shape, dtype))
                zero_outs.append(np.zeros(shape, dtype))
        self.in_names = list(in_names)
        self.out_names = out_names
        self.out_avals = out_avals
        self.zero_outs = zero_outs
        n_params = len(in_names)
        self.n_params = n_params

        all_names = in_names + out_names
        if partition_name is not None:
            all_names.append(partition_name)

        def _body(*args):
            operands = list(args)
            if partition_name is not None:
                operands.append(bass2jax.partition_id_tensor())
            outs = bass2jax._bass_exec_p.bind(
                *operands,
                out_avals=tuple(out_avals),
                in_names=tuple(all_names),
                out_names=tuple(out_names),
                lowering_input_output_aliases=(),
                sim_require_finite=True,
                sim_require_nnan=True,
                nc=nc,
            )
            return tuple(outs)

        devices = jax.devices()[:N_CORES]
        assert len(devices) == N_CORES
        self.mesh = Mesh(np.asarray(devices), ("core",))
        n_outs = len(out_names)
        in_specs = (PartitionSpec("core"),) * (n_params + n_outs)
        out_specs = (PartitionSpec("core"),) * n_outs
        donate = tuple(range(n_params, n_params + n_outs))
        self._fn = jax.jit(
            shard_map(
                _body, mesh=self.mesh, in_specs=in_specs, out_specs=out_specs,
                check_rep=False,
            ),
            donate_argnums=donate,
            keep_unused=True,
        )
        self._sharding = NamedSharding(self.mesh, PartitionSpec("core"))

    def put_inputs(self, in_maps):
        """Concat per-core inputs on axis 0 and move to device once."""
        concat = [
            np.concatenate([np.asarray(m[name]) for m in in_maps], axis=0)
            for name in self.in_names
        ]
        return [self._jax.device_put(a, self._sharding) for a in concat]

    def _zeros(self):
        return [
            np.zeros((N_CORES * z.shape[0], *z.shape[1:]), z.dtype)
            for z in self.zero_outs
        ]

    def execute(self, dev_inputs):
        outs = self._fn(*dev_inputs, *self._zeros())
        self._jax.block_until_ready(outs)
        return outs

    def run(self, in_maps):
        dev_inputs = self.put_inputs(in_maps)
        out_arrs = self.execute(dev_inputs)
        return [
            {
                name: np.asarray(out_arrs[i]).reshape(
                    N_CORES, *self.out_avals[i].shape
                )[c]
                for i, name in enumerate(self.out_names)
            }
            for c in range(N_CORES)
        ]


_RUNNER_CACHE = {}


def _get_runner():
    if "nc" not in _RUNNER_CACHE:
        _RUNNER_CACHE["nc"] = _Runner(build_nc())
    return _RUNNER_CACHE["nc"]


def _pack_x(xs, np_bf16):
    """[2048, 4096] -> bf16 hi/lo packed as [16, 128(p), 32(c)*128(m)]."""
    xh = xs.astype(np_bf16)
    xl = (xs - xh.astype(np.float32)).astype(np_bf16)
    out = []
    for a in (xh, xl):
        a4 = a.reshape(TOK_TILES, P, KC, P)          # [j, m, c, p]
        a4 = np.ascontiguousarray(a4.transpose(0, 3, 2, 1))  # [j, p, c, m]
        out.append(a4.reshape(TOK_TILES, P, KC * P))
    return out


def make_in_maps(hidden_states, weight, e_score_correction_bias):
    import ml_dtypes

    np_bf16 = ml_dtypes.bfloat16
    x = np.ascontiguousarray(np.asarray(hidden_states), dtype=np.float32)
    x = x.reshape(T_FULL, H)
    w = np.asarray(weight, dtype=np.float32)
    b = np.asarray(e_score_correction_bias, dtype=np.float32)

    wh = w.astype(np_bf16)
    wl = (w - wh.astype(np.float32)).astype(np_bf16)
    packed_w = []
    for a in (wh, wl):
        a3 = a.T.reshape(KC, P, E)                   # [c, p, e]
        a3 = np.ascontiguousarray(a3.transpose(1, 0, 2))  # [p, c, e]
        packed_w.append(a3.reshape(P, KC * E))
    whp, wlp = packed_w
    biasb = np.ascontiguousarray(np.broadcast_to(b, (P, E)))

    in_maps = []
    for i in range(N_CORES):
        xs = x[i * T_CORE:(i + 1) * T_CORE]
        xhp, xlp = _pack_x(xs, np_bf16)
        in_maps.append({
            "xh": xhp,
            "xl": xlp,
            "wh": whp,
            "wl": wlp,
            "biasb": biasb,
        })
    return in_maps


def kernel(hidden_states, weight, e_score_correction_bias):
    runner = _get_runner()
    results = runner.run(
        make_in_maps(hidden_states, weight, e_score_correction_bias)
    )
    topk_idx = np.concatenate(
        [r["idx_out"].astype(np.int32) for r in results], axis=0
    )
    topk_weight = np.concatenate([r["w_out"] for r in results], axis=0)
    return topk_idx, topk_weight
